# revision 11
# baseline (speedup 1.0000x reference)
"""MHA v5: algebraic K/V-projection elimination. No collectives.

Identities (per batch):
  S = (X Wq)(X Wk)^T = X M X^T,  M = Wq Wk^T   (M is seq-independent!)
  O = P (X Wv) = (P X) Wv = (U^T Wv),  U = X^T P^T  (contract keys first)

Per-core phases (own 1024 queries; keys rotated so own queries come first):
  1. M     [d,d']  = Wq Wk^T        : lhsT=wqT slices, rhs=wkT     128 mm
  2. Q'^T  [d',sq] = M^T X^T        : lhsT=m slices,   rhs=xt      128 mm
  3. S^T   [sk,sq] = X Q'^T         : lhsT=xt slices,  rhs=qt2     256 mm
     + exp (ACT, scale 1/8 fused) -> expS bf16
  4. rowsum (ones-matmul) + reciprocal broadcast
  5. U     [d,sq]  = X^T expS       : lhsT=xnat slices, rhs=expS   256 mm
  6. O^T   [e',sq] = Wv^T U         : lhsT=wv slices,  rhs=u       128 mm
     normalized by 1/den at the PSUM->SBUF multiply
  7. Y     [sq,f]  = O^T^T Wo       : lhsT=ot slices,  rhs=wo      128 mm

1024 big matmuls/core vs 1312 for the direct form (-22% PE work).
Host sends xt=X^T, xnat=X (both key-rotated for h=1), wqT=Wq^T, wkT=Wk^T,
wv, wo. All DMA lines >= 1KB.
"""

import numpy as np

import jax

import concourse.mybir as mybir
import concourse.tile as tile
from concourse import bacc


P = 128
D = 1024
S = 2048
SQ = 1024
B = 4
NCORES = 8
DT = D // P  # 8
SKT = S // P  # 16
NCH = 512
QCH = SQ // NCH  # 2
BF = mybir.dt.bfloat16
FP16 = mybir.dt.float16
F32 = mybir.dt.float32
SCALE = 0.125


def _build(reps=1, loop_reps=None):
    nc = bacc.Bacc("TRN2", debug=False, enable_asserts=False, num_devices=NCORES)

    xt_d = nc.dram_tensor("xt", [D, S], FP16, kind="ExternalInput").ap()
    xn_d = nc.dram_tensor("xn", [S, D], FP16, kind="ExternalInput").ap()
    wqt_d = nc.dram_tensor("wqt", [D, D], FP16, kind="ExternalInput").ap()
    wkt_d = nc.dram_tensor("wkt", [D, D], FP16, kind="ExternalInput").ap()
    wv_d = nc.dram_tensor("wv", [D, D], FP16, kind="ExternalInput").ap()
    wo_d = nc.dram_tensor("wo", [D, D], FP16, kind="ExternalInput").ap()
    y_d = nc.dram_tensor("y", [SQ, D], F32, kind="ExternalOutput").ap()

    with tile.TileContext(nc) as tc:
        with (
            tc.tile_pool(name="big", bufs=1) as big,
            tc.tile_pool(name="yst", bufs=2) as yst,
            tc.tile_pool(name="small", bufs=1) as small,
            tc.tile_pool(name="psmm", bufs=6, space="PSUM") as psmm,
            tc.tile_pool(name="psrow", bufs=2, space="PSUM") as psrow,
        ):
            import contextlib

            # timing loops run 2 bodies per HW-loop iteration: halves the
            # loop-edge cost and lets consecutive executions overlap as
            # straight-line dataflow
            if loop_reps:
                assert loop_reps % 2 == 0
                loop_ctx = tc.For_i(0, loop_reps // 2, 1)
                body_n = 2
            else:
                loop_ctx = contextlib.nullcontext()
                body_n = reps
            with loop_ctx:
                for _rep in range(body_n):
                    _body(nc, tc, big, yst, small, psmm, psrow,
                          xt_d, xn_d, wqt_d, wkt_d, wv_d, wo_d, y_d)

    nc.compile()
    return nc


def _body(nc, tc, big, yst, small, psmm, psrow,
          xt_d, xn_d, wqt_d, wkt_d, wv_d, wo_d, y_d):
    # ---- SBUF slots (per-partition KB): xt 32, xnat 32, wqT 16 (->wo),
    # wkT 16 (->expS_A), m 16 (->expS_B), qt2 16 (->u), wv 16, ot 16 = 160
    xt_s = big.tile([P, DT, S], FP16, tag="xt")
    xn_s = big.tile([P, SKT, D], FP16, tag="xn")
    wqt_s = big.tile([P, DT, D], FP16, tag="wqT")
    wkt_s = big.tile([P, DT, D], FP16, tag="wkT")
    m_s = big.tile([P, DT, D], FP16, tag="m")
    qt2_s = big.tile([P, DT, SQ], FP16, tag="qt2")
    wv_s = big.tile([P, DT, D], FP16, tag="wv")

    xt_r = xt_d.rearrange("(t p) s -> p t s", p=P)
    xn_r = xn_d.rearrange("(t p) d -> p t d", p=P)

    # weights first (M is the first phase), split in M-consumption order so
    # the first M group gates on ~2MB not 4MB; then xt (2KB lines), xnat
    wqt_r = wqt_d.rearrange("(t p) d -> p t d", p=P)
    wkt_r = wkt_d.rearrange("(t p) d -> p t d", p=P)
    HD = D // 2
    nc.sync.dma_start(wqt_s[:, :, 0:HD], wqt_r[:, :, 0:HD])
    nc.sync.dma_start(wkt_s[:, :, 0:HD], wkt_r[:, :, 0:HD])
    nc.sync.dma_start(wkt_s[:, :, HD:D], wkt_r[:, :, HD:D])
    nc.sync.dma_start(wqt_s[:, :, HD:D], wqt_r[:, :, HD:D])
    for cc in range(2):
        for dt in range(DT):
            nc.sync.dma_start(
                xt_s[:, dt, cc * SQ : (cc + 1) * SQ],
                xt_r[:, dt, cc * SQ : (cc + 1) * SQ],
            )
    nc.sync.dma_start(wv_s[:], wv_d.rearrange("(t p) e -> p t e", p=P))
    nc.sync.dma_start(xn_s[:], xn_r)

    # ---- 1. M = Wq Wk^T : out [d-tile, d'-chunk], accumulate over e
    for dtl in range(DT):
        pss = [psmm.tile([P, NCH], F32, tag="mm", name=f"ps{i}") for i in range(QCH)]
        for et, ch in [(e, c) for c in range(QCH) for e in range(DT)]:
            nc.tensor.matmul(
                pss[ch][:],
                wqt_s[:, et, dtl * P : (dtl + 1) * P],
                wkt_s[:, et, ch * NCH : (ch + 1) * NCH],
                start=(et == 0),
                stop=(et == DT - 1),
            )
        for ch in range(QCH):
            nc.any.tensor_copy(
                out=m_s[:, dtl, ch * NCH : (ch + 1) * NCH], in_=pss[ch][:]
            )

    # ---- 2. Q'^T = M^T X^T : out [d'-tile, sq-chunk], accumulate over d
    for dtl in range(DT):
        pss = [psmm.tile([P, NCH], F32, tag="mm", name=f"ps{i}") for i in range(QCH)]
        for dt, ch in [(d, c) for c in range(QCH) for d in range(DT)]:
            nc.tensor.matmul(
                pss[ch][:],
                m_s[:, dt, dtl * P : (dtl + 1) * P],
                xt_s[:, dt, ch * NCH : (ch + 1) * NCH],
                start=(dt == 0),
                stop=(dt == DT - 1),
            )
        for ch in range(QCH):
            nc.any.tensor_copy(
                out=qt2_s[:, dtl, ch * NCH : (ch + 1) * NCH], in_=pss[ch][:]
            )

    # m dead -> wo loads into its slot; wkT dead -> expS_A; (wqT -> wo? no:
    # wqT slot hosts wo NEXT rep; this rep wo goes into the m slot)
    expS_A = big.tile([P, SKT // 2, SQ], BF, tag="wkT")
    expS_B = big.tile([P, SKT // 2, SQ], BF, tag="wqT")
    wo_s = big.tile([P, DT, D], FP16, tag="m")
    nc.sync.dma_start(wo_s[:], wo_d.rearrange("(t p) f -> p t f", p=P))

    def _expS(skt):
        return expS_A[:, skt] if skt < SKT // 2 else expS_B[:, skt - SKT // 2]

    # ---- 3. S^T = X Q'^T : out [sk-tile, sq-chunk], accumulate over d'
    for skt in range(SKT):
        pss = [psmm.tile([P, NCH], F32, tag="mm", name=f"ps{i}") for i in range(QCH)]
        for dt, ch in [(d, c) for d in range(DT) for c in range(QCH)]:
            nc.tensor.matmul(
                pss[ch][:],
                xt_s[:, dt, skt * P : (skt + 1) * P],
                qt2_s[:, dt, ch * NCH : (ch + 1) * NCH],
                start=(dt == 0),
                stop=(dt == DT - 1),
            )
        for ch in range(QCH):
            nc.scalar.activation(
                _expS(skt)[:, ch * NCH : (ch + 1) * NCH],
                pss[ch][:],
                mybir.ActivationFunctionType.Exp,
                scale=SCALE,
            )

    # ---- 4. row sums + reciprocal broadcast
    ones = small.tile([P, 1], BF)
    nc.any.memset(ones[:], 1.0)
    row = small.tile([1, SQ], F32)
    for ch in range(QCH):
        psr = psrow.tile([1, NCH], F32, tag="row")
        for skt in range(SKT):
            nc.tensor.matmul(
                psr[:],
                ones[:],
                _expS(skt)[:, ch * NCH : (ch + 1) * NCH],
                start=(skt == 0),
                stop=(skt == SKT - 1),
            )
        nc.any.tensor_copy(out=row[:, ch * NCH : (ch + 1) * NCH], in_=psr[:])
    rrow = small.tile([1, SQ], F32)
    ones1 = small.tile([1, P], F32)
    nc.any.memset(ones1[:], 1.0)
    recip_rep = small.tile([P, SQ], F32)

    def _recip_block():
        nc.vector.reciprocal(rrow[:], row[:])
        for ch in range(QCH):
            psr2 = psmm.tile([P, NCH], F32, tag="mm", name="psrep")
            nc.tensor.matmul(
                psr2[:],
                ones1[:],
                rrow[:, ch * NCH : (ch + 1) * NCH],
                start=True,
                stop=True,
            )
            nc.any.tensor_copy(
                out=recip_rep[:, ch * NCH : (ch + 1) * NCH], in_=psr2[:]
            )

    # ---- 5. U = X^T expS : out [d-tile, sq-chunk], accumulate over keys
    u_s = big.tile([P, DT, SQ], BF, tag="qt2")
    for dtl in range(DT):
        pss = [psmm.tile([P, NCH], F32, tag="mm", name=f"ps{i}") for i in range(QCH)]
        for skt, ch in [(s_, c) for s_ in range(SKT) for c in range(QCH)]:
            nc.tensor.matmul(
                pss[ch][:],
                xn_s[:, skt, dtl * P : (dtl + 1) * P],
                _expS(skt)[:, ch * NCH : (ch + 1) * NCH],
                start=(skt == 0),
                stop=(skt == SKT - 1),
            )
        if dtl == 0:
            _recip_block()
        for ch in range(QCH):
            nc.any.tensor_copy(
                out=u_s[:, dtl, ch * NCH : (ch + 1) * NCH], in_=pss[ch][:]
            )

    # ---- 6. O^T = Wv^T U : out [e'-tile, sq-chunk], accumulate over d;
    # 1/den fused at the PSUM->SBUF multiply
    ot_s = big.tile([P, DT, SQ], FP16, tag="ot")
    for et in range(DT):
        pss = [psmm.tile([P, NCH], F32, tag="mm", name=f"ps{i}") for i in range(QCH)]
        for dt, ch in [(d, c) for c in range(QCH) for d in range(DT)]:
            nc.tensor.matmul(
                pss[ch][:],
                wv_s[:, dt, et * P : (et + 1) * P],
                u_s[:, dt, ch * NCH : (ch + 1) * NCH],
                start=(dt == 0),
                stop=(dt == DT - 1),
            )
        for ch in range(QCH):
            nc.vector.tensor_mul(
                out=ot_s[:, et, ch * NCH : (ch + 1) * NCH],
                in0=pss[ch][:],
                in1=recip_rep[:, ch * NCH : (ch + 1) * NCH],
            )

    # ---- 7. Y = O^T^T Wo
    SQT = SQ // P
    FCH = D // NCH
    for sqt in range(SQT):
        pss = [psmm.tile([P, NCH], F32, tag="mm", name=f"ps{i}") for i in range(FCH)]
        for et, ch in [(e, c) for c in range(FCH) for e in range(DT)]:
            nc.tensor.matmul(
                pss[ch][:],
                ot_s[:, et, sqt * P : (sqt + 1) * P],
                wo_s[:, et, ch * NCH : (ch + 1) * NCH],
                start=(et == 0),
                stop=(et == DT - 1),
            )
        y_stage = yst.tile([P, D], F32, tag="y")
        for ch in range(FCH):
            nc.any.tensor_copy(out=y_stage[:, ch * NCH : (ch + 1) * NCH], in_=pss[ch][:])
        nc.sync.dma_start(y_d[sqt * P : (sqt + 1) * P, :], y_stage[:])


# ---------------------------------------------------------------------------
# PJRT runner (axon): jit once per process, chain `reps` executions.
# ---------------------------------------------------------------------------

def _make_runner(nc, n_cores, reps=1):
    from concourse.bass2jax import (
        _bass_exec_p,
        install_neuronx_cc_hook,
        partition_id_tensor,
    )
    from jax.sharding import Mesh, PartitionSpec
    from jax.experimental.shard_map import shard_map

    install_neuronx_cc_hook()
    partition_name = nc.partition_id_tensor.name if nc.partition_id_tensor else None

    in_names, out_names, out_avals, zero_outs = [], [], [], []
    for alloc in nc.m.functions[0].allocations:
        if not isinstance(alloc, mybir.MemoryLocationSet):
            continue
        name = alloc.memorylocations[0].name
        if alloc.kind == "ExternalInput":
            if name != partition_name:
                in_names.append(name)
        elif alloc.kind == "ExternalOutput":
            shape = tuple(alloc.tensor_shape)
            dtype = mybir.dt.np(alloc.dtype)
            out_names.append(name)
            out_avals.append(jax.core.ShapedArray(shape, dtype))
            zero_outs.append(np.zeros(shape, dtype))
    n_params = len(in_names)
    n_outs = len(out_avals)
    all_in_names = list(in_names) + list(out_names)
    if partition_name is not None:
        all_in_names.append(partition_name)

    def _body(*args):
        operands = list(args)
        pid = [partition_id_tensor()] if partition_name is not None else []
        outs = None
        for _ in range(reps):
            outs = _bass_exec_p.bind(
                *operands,
                *pid,
                out_avals=tuple(out_avals),
                in_names=tuple(all_in_names),
                out_names=tuple(out_names),
                lowering_input_output_aliases=(),
                sim_require_finite=True,
                sim_require_nnan=True,
                nc=nc,
            )
        return tuple(outs)

    devices = jax.devices()[:n_cores]
    mesh = Mesh(np.asarray(devices), ("core",))
    in_specs = (PartitionSpec("core"),) * (n_params + n_outs)
    out_specs = (PartitionSpec("core"),) * n_outs
    fn = jax.jit(
        shard_map(
            _body, mesh=mesh, in_specs=in_specs, out_specs=out_specs, check_rep=False
        )
    )

    def run(in_maps):
        per_core = [[np.asarray(m[name]) for name in in_names] for m in in_maps]
        concat_in = [
            np.ascontiguousarray(
                np.concatenate([per_core[c][i] for c in range(n_cores)], axis=0)
            )
            for i in range(n_params)
        ]
        concat_zeros = [
            np.zeros((n_cores * z.shape[0], *z.shape[1:]), z.dtype) for z in zero_outs
        ]
        out_arrs = fn(*concat_in, *concat_zeros)
        jax.block_until_ready(out_arrs)
        return [
            {
                name: np.asarray(out_arrs[i]).reshape(n_cores, *out_avals[i].shape)[c]
                for i, name in enumerate(out_names)
            }
            for c in range(n_cores)
        ]

    run.fn = fn
    run.in_names = in_names
    run.out_names = out_names
    run.zero_outs = zero_outs
    run.n_cores = n_cores
    return run


_CACHE = {}


def _get_runner(reps=1):
    key = ("runner", reps)
    if key not in _CACHE:
        _CACHE[key] = _make_runner(_build(reps=reps), NCORES)
    return _CACHE[key]


def _prep_in_maps(inputs, Wq, Wk, Wv, Wo):
    f16 = np.float16
    w = {
        "wqt": np.ascontiguousarray(np.asarray(Wq).T).astype(f16),
        "wkt": np.ascontiguousarray(np.asarray(Wk).T).astype(f16),
        "wv": np.ascontiguousarray(Wv).astype(f16),
        "wo": np.ascontiguousarray(Wo).astype(f16),
    }
    in_maps = []
    for c in range(NCORES):
        b, h = divmod(c, 2)
        xb = np.asarray(inputs[b])
        if h == 1:
            # own queries first; keys rotated identically in both layouts
            xb = np.concatenate([xb[SQ:], xb[:SQ]], axis=0)
        in_maps.append(
            {
                "xt": np.ascontiguousarray(xb.T).astype(f16),
                "xn": np.ascontiguousarray(xb).astype(f16),
                **w,
            }
        )
    return in_maps


def kernel(inputs, Wq, Wk, Wv, Wo):
    inputs = np.asarray(inputs, dtype=np.float32)
    run = _get_runner()
    in_maps = _prep_in_maps(inputs, Wq, Wk, Wv, Wo)
    res = run(in_maps)
    out = np.empty((B, S, D), dtype=np.float32)
    for c in range(NCORES):
        b, h = divmod(c, 2)
        out[b, h * SQ : (h + 1) * SQ] = res[c]["y"]
    return out


# revision 12
# speedup vs baseline: 1.0161x; 1.0161x over previous
"""MHA v5: algebraic K/V-projection elimination. No collectives.

Identities (per batch):
  S = (X Wq)(X Wk)^T = X M X^T,  M = Wq Wk^T   (M is seq-independent!)
  O = P (X Wv) = (P X) Wv = (U^T Wv),  U = X^T P^T  (contract keys first)

Per-core phases (own 1024 queries; keys rotated so own queries come first):
  1. M     [d,d']  = Wq Wk^T        : lhsT=wqT slices, rhs=wkT     128 mm
  2. Q'^T  [d',sq] = M^T X^T        : lhsT=m slices,   rhs=xt      128 mm
  3. S^T   [sk,sq] = X Q'^T         : lhsT=xt slices,  rhs=qt2     256 mm
     + exp (ACT, scale 1/8 fused) -> expS bf16
  4. rowsum (ones-matmul) + reciprocal broadcast
  5. U     [d,sq]  = X^T expS       : lhsT=xnat slices, rhs=expS   256 mm
  6. O^T   [e',sq] = Wv^T U         : lhsT=wv slices,  rhs=u       128 mm
     normalized by 1/den at the PSUM->SBUF multiply
  7. Y     [sq,f]  = O^T^T Wo       : lhsT=ot slices,  rhs=wo      128 mm

1024 big matmuls/core vs 1312 for the direct form (-22% PE work).
Host sends xt=X^T, xnat=X (both key-rotated for h=1), wqT=Wq^T, wkT=Wk^T,
wv, wo. All DMA lines >= 1KB.
"""

import numpy as np

import jax

import concourse.mybir as mybir
import concourse.tile as tile
from concourse import bacc


P = 128
D = 1024
S = 2048
SQ = 1024
B = 4
NCORES = 8
DT = D // P  # 8
SKT = S // P  # 16
NCH = 512
QCH = SQ // NCH  # 2
BF = mybir.dt.bfloat16
FP16 = mybir.dt.float16
F32 = mybir.dt.float32
SCALE = 0.125


def _build(reps=1, loop_reps=None):
    nc = bacc.Bacc("TRN2", debug=False, enable_asserts=False, num_devices=NCORES)

    xt_d = nc.dram_tensor("xt", [D, S], FP16, kind="ExternalInput").ap()
    xn_d = nc.dram_tensor("xn", [S, D], FP16, kind="ExternalInput").ap()
    wqt_d = nc.dram_tensor("wqt", [D, D], FP16, kind="ExternalInput").ap()
    wkt_d = nc.dram_tensor("wkt", [D, D], FP16, kind="ExternalInput").ap()
    wv_d = nc.dram_tensor("wv", [D, D], FP16, kind="ExternalInput").ap()
    wo_d = nc.dram_tensor("wo", [D, D], FP16, kind="ExternalInput").ap()
    y_d = nc.dram_tensor("y", [SQ, D], F32, kind="ExternalOutput").ap()

    with tile.TileContext(nc) as tc:
        with (
            tc.tile_pool(name="big", bufs=1) as big,
            tc.tile_pool(name="yst", bufs=2) as yst,
            tc.tile_pool(name="small", bufs=1) as small,
            tc.tile_pool(name="psmm", bufs=6, space="PSUM") as psmm,
            tc.tile_pool(name="psrow", bufs=2, space="PSUM") as psrow,
        ):
            import contextlib

            # timing loops run 2 bodies per HW-loop iteration: halves the
            # loop-edge cost and lets consecutive executions overlap as
            # straight-line dataflow
            if loop_reps:
                assert loop_reps % 2 == 0
                loop_ctx = tc.For_i(0, loop_reps // 2, 1)
                body_n = 2
            else:
                loop_ctx = contextlib.nullcontext()
                body_n = reps
            with loop_ctx:
                for _rep in range(body_n):
                    _body(nc, tc, big, yst, small, psmm, psrow,
                          xt_d, xn_d, wqt_d, wkt_d, wv_d, wo_d, y_d)

    nc.compile()
    return nc


def _body(nc, tc, big, yst, small, psmm, psrow,
          xt_d, xn_d, wqt_d, wkt_d, wv_d, wo_d, y_d):
    # ---- SBUF slots (per-partition KB): xt 32, xnat 32, wqT 16 (->wo),
    # wkT 16 (->expS_A), m 16 (->expS_B), qt2 16 (->u), wv 16, ot 16 = 160
    xt_s = big.tile([P, DT, S], FP16, tag="xt")
    xn_s = big.tile([P, SKT, D], FP16, tag="xn")
    wqt_s = big.tile([P, DT, D], FP16, tag="wqT")
    wkt_s = big.tile([P, DT, D], FP16, tag="wkT")
    m_s = big.tile([P, DT, D], FP16, tag="m")
    qt2_s = big.tile([P, DT, SQ], FP16, tag="qt2")
    wv_s = big.tile([P, DT, D], FP16, tag="wv")

    xt_r = xt_d.rearrange("(t p) s -> p t s", p=P)
    xn_r = xn_d.rearrange("(t p) d -> p t d", p=P)

    # weights first (M is the first phase), split in M-consumption order so
    # the first M group gates on ~2MB not 4MB; then xt (2KB lines), xnat
    wqt_r = wqt_d.rearrange("(t p) d -> p t d", p=P)
    wkt_r = wkt_d.rearrange("(t p) d -> p t d", p=P)
    HD = D // 2
    nc.sync.dma_start(wqt_s[:, :, 0:HD], wqt_r[:, :, 0:HD])
    nc.sync.dma_start(wkt_s[:, :, 0:HD], wkt_r[:, :, 0:HD])
    nc.sync.dma_start(wkt_s[:, :, HD:D], wkt_r[:, :, HD:D])
    nc.sync.dma_start(wqt_s[:, :, HD:D], wqt_r[:, :, HD:D])
    for cc in range(2):
        for dt in range(DT):
            nc.sync.dma_start(
                xt_s[:, dt, cc * SQ : (cc + 1) * SQ],
                xt_r[:, dt, cc * SQ : (cc + 1) * SQ],
            )
    nc.sync.dma_start(wv_s[:], wv_d.rearrange("(t p) e -> p t e", p=P))
    nc.sync.dma_start(xn_s[:], xn_r)

    # ---- 1. M = Wq Wk^T : out [d-tile, d'-chunk], accumulate over e
    for dtl in range(DT):
        pss = [psmm.tile([P, NCH], F32, tag="mm", name=f"ps{i}") for i in range(QCH)]
        for et, ch in [(e, c) for c in range(QCH) for e in range(DT)]:
            nc.tensor.matmul(
                pss[ch][:],
                wqt_s[:, et, dtl * P : (dtl + 1) * P],
                wkt_s[:, et, ch * NCH : (ch + 1) * NCH],
                start=(et == 0),
                stop=(et == DT - 1),
            )
        for ch in range(QCH):
            nc.any.tensor_copy(
                out=m_s[:, dtl, ch * NCH : (ch + 1) * NCH], in_=pss[ch][:]
            )

    # ---- 2. Q'^T = M^T X^T : out [d'-tile, sq-chunk], accumulate over d
    for dtl in range(DT):
        pss = [psmm.tile([P, NCH], F32, tag="mm", name=f"ps{i}") for i in range(QCH)]
        for dt, ch in [(d, c) for c in range(QCH) for d in range(DT)]:
            nc.tensor.matmul(
                pss[ch][:],
                m_s[:, dt, dtl * P : (dtl + 1) * P],
                xt_s[:, dt, ch * NCH : (ch + 1) * NCH],
                start=(dt == 0),
                stop=(dt == DT - 1),
            )
        for ch in range(QCH):
            nc.any.tensor_copy(
                out=qt2_s[:, dtl, ch * NCH : (ch + 1) * NCH], in_=pss[ch][:]
            )

    # m dead -> wo loads into its slot; wkT dead -> expS_A; (wqT -> wo? no:
    # wqT slot hosts wo NEXT rep; this rep wo goes into the m slot)
    expS_A = big.tile([P, SKT // 2, SQ], BF, tag="wkT")
    expS_B = big.tile([P, SKT // 2, SQ], BF, tag="wqT")
    wo_s = big.tile([P, DT, D], FP16, tag="m")
    nc.sync.dma_start(wo_s[:], wo_d.rearrange("(t p) f -> p t f", p=P))

    def _expS(skt):
        return expS_A[:, skt] if skt < SKT // 2 else expS_B[:, skt - SKT // 2]

    # ---- 3. S^T = X Q'^T : out [sk-tile, sq-chunk], accumulate over d'
    for skt in range(SKT):
        pss = [psmm.tile([P, NCH], F32, tag="mm", name=f"ps{i}") for i in range(QCH)]
        for dt, ch in [(d, c) for c in range(QCH) for d in range(DT)]:
            nc.tensor.matmul(
                pss[ch][:],
                xt_s[:, dt, skt * P : (skt + 1) * P],
                qt2_s[:, dt, ch * NCH : (ch + 1) * NCH],
                start=(dt == 0),
                stop=(dt == DT - 1),
            )
        for ch in range(QCH):
            nc.scalar.activation(
                _expS(skt)[:, ch * NCH : (ch + 1) * NCH],
                pss[ch][:],
                mybir.ActivationFunctionType.Exp,
                scale=SCALE,
            )

    # ---- 4. row sums + reciprocal broadcast
    ones = small.tile([P, 1], BF)
    nc.any.memset(ones[:], 1.0)
    row = small.tile([1, SQ], F32)
    for ch in range(QCH):
        psr = psrow.tile([1, NCH], F32, tag="row")
        for skt in range(SKT):
            nc.tensor.matmul(
                psr[:],
                ones[:],
                _expS(skt)[:, ch * NCH : (ch + 1) * NCH],
                start=(skt == 0),
                stop=(skt == SKT - 1),
            )
        nc.any.tensor_copy(out=row[:, ch * NCH : (ch + 1) * NCH], in_=psr[:])
    rrow = small.tile([1, SQ], F32)
    ones1 = small.tile([1, P], F32)
    nc.any.memset(ones1[:], 1.0)
    recip_rep = small.tile([P, SQ], F32)

    def _recip_block():
        nc.vector.reciprocal(rrow[:], row[:])
        for ch in range(QCH):
            psr2 = psmm.tile([P, NCH], F32, tag="mm", name="psrep")
            nc.tensor.matmul(
                psr2[:],
                ones1[:],
                rrow[:, ch * NCH : (ch + 1) * NCH],
                start=True,
                stop=True,
            )
            nc.any.tensor_copy(
                out=recip_rep[:, ch * NCH : (ch + 1) * NCH], in_=psr2[:]
            )

    # ---- 5. U = X^T expS : out [d-tile, sq-chunk], accumulate over keys
    u_s = big.tile([P, DT, SQ], BF, tag="qt2")
    for dtl in range(DT):
        pss = [psmm.tile([P, NCH], F32, tag="mm", name=f"ps{i}") for i in range(QCH)]
        for skt, ch in [(s_, c) for c in range(QCH) for s_ in range(SKT)]:
            nc.tensor.matmul(
                pss[ch][:],
                xn_s[:, skt, dtl * P : (dtl + 1) * P],
                _expS(skt)[:, ch * NCH : (ch + 1) * NCH],
                start=(skt == 0),
                stop=(skt == SKT - 1),
            )
        if dtl == 0:
            _recip_block()
        for ch in range(QCH):
            nc.any.tensor_copy(
                out=u_s[:, dtl, ch * NCH : (ch + 1) * NCH], in_=pss[ch][:]
            )

    # ---- 6. O^T = Wv^T U : out [e'-tile, sq-chunk], accumulate over d;
    # 1/den fused at the PSUM->SBUF multiply
    ot_s = big.tile([P, DT, SQ], FP16, tag="ot")
    for et in range(DT):
        pss = [psmm.tile([P, NCH], F32, tag="mm", name=f"ps{i}") for i in range(QCH)]
        for dt, ch in [(d, c) for c in range(QCH) for d in range(DT)]:
            nc.tensor.matmul(
                pss[ch][:],
                wv_s[:, dt, et * P : (et + 1) * P],
                u_s[:, dt, ch * NCH : (ch + 1) * NCH],
                start=(dt == 0),
                stop=(dt == DT - 1),
            )
        for ch in range(QCH):
            nc.vector.tensor_mul(
                out=ot_s[:, et, ch * NCH : (ch + 1) * NCH],
                in0=pss[ch][:],
                in1=recip_rep[:, ch * NCH : (ch + 1) * NCH],
            )

    # ---- 7. Y = O^T^T Wo
    SQT = SQ // P
    FCH = D // NCH
    for sqt in range(SQT):
        pss = [psmm.tile([P, NCH], F32, tag="mm", name=f"ps{i}") for i in range(FCH)]
        for et, ch in [(e, c) for c in range(FCH) for e in range(DT)]:
            nc.tensor.matmul(
                pss[ch][:],
                ot_s[:, et, sqt * P : (sqt + 1) * P],
                wo_s[:, et, ch * NCH : (ch + 1) * NCH],
                start=(et == 0),
                stop=(et == DT - 1),
            )
        y_stage = yst.tile([P, D], F32, tag="y")
        for ch in range(FCH):
            nc.any.tensor_copy(out=y_stage[:, ch * NCH : (ch + 1) * NCH], in_=pss[ch][:])
        nc.sync.dma_start(y_d[sqt * P : (sqt + 1) * P, :], y_stage[:])


# ---------------------------------------------------------------------------
# PJRT runner (axon): jit once per process, chain `reps` executions.
# ---------------------------------------------------------------------------

def _make_runner(nc, n_cores, reps=1):
    from concourse.bass2jax import (
        _bass_exec_p,
        install_neuronx_cc_hook,
        partition_id_tensor,
    )
    from jax.sharding import Mesh, PartitionSpec
    from jax.experimental.shard_map import shard_map

    install_neuronx_cc_hook()
    partition_name = nc.partition_id_tensor.name if nc.partition_id_tensor else None

    in_names, out_names, out_avals, zero_outs = [], [], [], []
    for alloc in nc.m.functions[0].allocations:
        if not isinstance(alloc, mybir.MemoryLocationSet):
            continue
        name = alloc.memorylocations[0].name
        if alloc.kind == "ExternalInput":
            if name != partition_name:
                in_names.append(name)
        elif alloc.kind == "ExternalOutput":
            shape = tuple(alloc.tensor_shape)
            dtype = mybir.dt.np(alloc.dtype)
            out_names.append(name)
            out_avals.append(jax.core.ShapedArray(shape, dtype))
            zero_outs.append(np.zeros(shape, dtype))
    n_params = len(in_names)
    n_outs = len(out_avals)
    all_in_names = list(in_names) + list(out_names)
    if partition_name is not None:
        all_in_names.append(partition_name)

    def _body(*args):
        operands = list(args)
        pid = [partition_id_tensor()] if partition_name is not None else []
        outs = None
        for _ in range(reps):
            outs = _bass_exec_p.bind(
                *operands,
                *pid,
                out_avals=tuple(out_avals),
                in_names=tuple(all_in_names),
                out_names=tuple(out_names),
                lowering_input_output_aliases=(),
                sim_require_finite=True,
                sim_require_nnan=True,
                nc=nc,
            )
        return tuple(outs)

    devices = jax.devices()[:n_cores]
    mesh = Mesh(np.asarray(devices), ("core",))
    in_specs = (PartitionSpec("core"),) * (n_params + n_outs)
    out_specs = (PartitionSpec("core"),) * n_outs
    fn = jax.jit(
        shard_map(
            _body, mesh=mesh, in_specs=in_specs, out_specs=out_specs, check_rep=False
        )
    )

    def run(in_maps):
        per_core = [[np.asarray(m[name]) for name in in_names] for m in in_maps]
        concat_in = [
            np.ascontiguousarray(
                np.concatenate([per_core[c][i] for c in range(n_cores)], axis=0)
            )
            for i in range(n_params)
        ]
        concat_zeros = [
            np.zeros((n_cores * z.shape[0], *z.shape[1:]), z.dtype) for z in zero_outs
        ]
        out_arrs = fn(*concat_in, *concat_zeros)
        jax.block_until_ready(out_arrs)
        return [
            {
                name: np.asarray(out_arrs[i]).reshape(n_cores, *out_avals[i].shape)[c]
                for i, name in enumerate(out_names)
            }
            for c in range(n_cores)
        ]

    run.fn = fn
    run.in_names = in_names
    run.out_names = out_names
    run.zero_outs = zero_outs
    run.n_cores = n_cores
    return run


_CACHE = {}


def _get_runner(reps=1):
    key = ("runner", reps)
    if key not in _CACHE:
        _CACHE[key] = _make_runner(_build(reps=reps), NCORES)
    return _CACHE[key]


def _prep_in_maps(inputs, Wq, Wk, Wv, Wo):
    f16 = np.float16
    w = {
        "wqt": np.ascontiguousarray(np.asarray(Wq).T).astype(f16),
        "wkt": np.ascontiguousarray(np.asarray(Wk).T).astype(f16),
        "wv": np.ascontiguousarray(Wv).astype(f16),
        "wo": np.ascontiguousarray(Wo).astype(f16),
    }
    in_maps = []
    for c in range(NCORES):
        b, h = divmod(c, 2)
        xb = np.asarray(inputs[b])
        if h == 1:
            # own queries first; keys rotated identically in both layouts
            xb = np.concatenate([xb[SQ:], xb[:SQ]], axis=0)
        in_maps.append(
            {
                "xt": np.ascontiguousarray(xb.T).astype(f16),
                "xn": np.ascontiguousarray(xb).astype(f16),
                **w,
            }
        )
    return in_maps


def kernel(inputs, Wq, Wk, Wv, Wo):
    inputs = np.asarray(inputs, dtype=np.float32)
    run = _get_runner()
    in_maps = _prep_in_maps(inputs, Wq, Wk, Wv, Wo)
    res = run(in_maps)
    out = np.empty((B, S, D), dtype=np.float32)
    for c in range(NCORES):
        b, h = divmod(c, 2)
        out[b, h * SQ : (h + 1) * SQ] = res[c]["y"]
    return out


# revision 14
# speedup vs baseline: 1.0184x; 1.0022x over previous
"""MHA v5: algebraic K/V-projection elimination. No collectives.

Identities (per batch):
  S = (X Wq)(X Wk)^T = X M X^T,  M = Wq Wk^T   (M is seq-independent!)
  O = P (X Wv) = (P X) Wv = (U^T Wv),  U = X^T P^T  (contract keys first)

Per-core phases (own 1024 queries; keys rotated so own queries come first):
  1. M     [d,d']  = Wq Wk^T        : lhsT=wqT slices, rhs=wkT     128 mm
  2. Q'^T  [d',sq] = M^T X^T        : lhsT=m slices,   rhs=xt      128 mm
  3. S^T   [sk,sq] = X Q'^T         : lhsT=xt slices,  rhs=qt2     256 mm
     + exp (ACT, scale 1/8 fused) -> expS bf16
  4. rowsum (ones-matmul) + reciprocal broadcast
  5. U     [d,sq]  = X^T expS       : lhsT=xnat slices, rhs=expS   256 mm
  6. O^T   [e',sq] = Wv^T U         : lhsT=wv slices,  rhs=u       128 mm
     normalized by 1/den at the PSUM->SBUF multiply
  7. Y     [sq,f]  = O^T^T Wo       : lhsT=ot slices,  rhs=wo      128 mm

1024 big matmuls/core vs 1312 for the direct form (-22% PE work).
Host sends xt=X^T, xnat=X (both key-rotated for h=1), wqT=Wq^T, wkT=Wk^T,
wv, wo. All DMA lines >= 1KB.
"""

import numpy as np

import jax

import concourse.mybir as mybir
import concourse.tile as tile
from concourse import bacc


P = 128
D = 1024
S = 2048
SQ = 1024
B = 4
NCORES = 8
DT = D // P  # 8
SKT = S // P  # 16
NCH = 512
QCH = SQ // NCH  # 2
BF = mybir.dt.bfloat16
FP16 = mybir.dt.float16
F32 = mybir.dt.float32
SCALE = 0.125


def _build(reps=1, loop_reps=None):
    nc = bacc.Bacc("TRN2", debug=False, enable_asserts=False, num_devices=NCORES)

    xt_d = nc.dram_tensor("xt", [D, S], FP16, kind="ExternalInput").ap()
    xn_d = nc.dram_tensor("xn", [S, D], FP16, kind="ExternalInput").ap()
    wqt_d = nc.dram_tensor("wqt", [D, D], FP16, kind="ExternalInput").ap()
    wkt_d = nc.dram_tensor("wkt", [D, D], FP16, kind="ExternalInput").ap()
    wv_d = nc.dram_tensor("wv", [D, D], FP16, kind="ExternalInput").ap()
    wo_d = nc.dram_tensor("wo", [D, D], FP16, kind="ExternalInput").ap()
    y_d = nc.dram_tensor("y", [SQ, D], F32, kind="ExternalOutput").ap()

    with tile.TileContext(nc) as tc:
        with (
            tc.tile_pool(name="big", bufs=1) as big,
            tc.tile_pool(name="yst", bufs=2) as yst,
            tc.tile_pool(name="small", bufs=1) as small,
            tc.tile_pool(name="psmm", bufs=6, space="PSUM") as psmm,
            tc.tile_pool(name="psrow", bufs=2, space="PSUM") as psrow,
        ):
            import contextlib

            # timing loops run 2 bodies per HW-loop iteration: halves the
            # loop-edge cost and lets consecutive executions overlap as
            # straight-line dataflow
            if loop_reps:
                assert loop_reps % 2 == 0
                loop_ctx = tc.For_i(0, loop_reps // 2, 1)
                body_n = 2
            else:
                loop_ctx = contextlib.nullcontext()
                body_n = reps
            with loop_ctx:
                for _rep in range(body_n):
                    _body(nc, tc, big, yst, small, psmm, psrow,
                          xt_d, xn_d, wqt_d, wkt_d, wv_d, wo_d, y_d)

    nc.compile()
    return nc


def _body(nc, tc, big, yst, small, psmm, psrow,
          xt_d, xn_d, wqt_d, wkt_d, wv_d, wo_d, y_d):
    # ---- SBUF slots (per-partition KB): xt 32, xnat 32, wqT 16 (->wo),
    # wkT 16 (->expS_A), m 16 (->expS_B), qt2 16 (->u), wv 16, ot 16 = 160
    xt_s = big.tile([P, DT, S], FP16, tag="xt")
    xn_s = big.tile([P, SKT, D], FP16, tag="xn")
    wqt_s = big.tile([P, DT, D], FP16, tag="wqT")
    wkt_s = big.tile([P, DT, D], FP16, tag="wkT")
    m_s = big.tile([P, DT, D], FP16, tag="m")
    qt2_s = big.tile([P, DT, SQ], FP16, tag="qt2")
    wv_s = big.tile([P, DT, D], FP16, tag="wv")

    xt_r = xt_d.rearrange("(t p) s -> p t s", p=P)
    xn_r = xn_d.rearrange("(t p) d -> p t d", p=P)

    # weights first (M is the first phase), split in M-consumption order so
    # the first M group gates on ~2MB not 4MB; then xt (2KB lines), xnat
    wqt_r = wqt_d.rearrange("(t p) d -> p t d", p=P)
    wkt_r = wkt_d.rearrange("(t p) d -> p t d", p=P)
    HD = D // 2
    nc.sync.dma_start(wqt_s[:, :, 0:HD], wqt_r[:, :, 0:HD])
    nc.sync.dma_start(wkt_s[:, :, 0:HD], wkt_r[:, :, 0:HD])
    nc.sync.dma_start(wkt_s[:, :, HD:D], wkt_r[:, :, HD:D])
    nc.sync.dma_start(wqt_s[:, :, HD:D], wqt_r[:, :, HD:D])
    for cc in range(2):
        for dt in range(DT):
            nc.sync.dma_start(
                xt_s[:, dt, cc * SQ : (cc + 1) * SQ],
                xt_r[:, dt, cc * SQ : (cc + 1) * SQ],
            )
    nc.sync.dma_start(wv_s[:], wv_d.rearrange("(t p) e -> p t e", p=P))
    nc.sync.dma_start(xn_s[:], xn_r)

    # ---- 1. M = Wq Wk^T : out [d-tile, d'-chunk], accumulate over e
    for dtl in range(DT):
        pss = [psmm.tile([P, NCH], F32, tag="mm", name=f"ps{i}") for i in range(QCH)]
        for et, ch in [(e, c) for c in range(QCH) for e in range(DT)]:
            nc.tensor.matmul(
                pss[ch][:],
                wqt_s[:, et, dtl * P : (dtl + 1) * P],
                wkt_s[:, et, ch * NCH : (ch + 1) * NCH],
                start=(et == 0),
                stop=(et == DT - 1),
            )
        for ch in range(QCH):
            nc.any.tensor_copy(
                out=m_s[:, dtl, ch * NCH : (ch + 1) * NCH], in_=pss[ch][:]
            )

    # ---- 2. Q'^T = M^T X^T : out [d'-tile, sq-chunk], accumulate over d
    for dtl in range(DT):
        pss = [psmm.tile([P, NCH], F32, tag="mm", name=f"ps{i}") for i in range(QCH)]
        for dt, ch in [(d, c) for c in range(QCH) for d in range(DT)]:
            nc.tensor.matmul(
                pss[ch][:],
                m_s[:, dt, dtl * P : (dtl + 1) * P],
                xt_s[:, dt, ch * NCH : (ch + 1) * NCH],
                start=(dt == 0),
                stop=(dt == DT - 1),
            )
        for ch in range(QCH):
            nc.any.tensor_copy(
                out=qt2_s[:, dtl, ch * NCH : (ch + 1) * NCH], in_=pss[ch][:]
            )

    # m dead -> wo loads into its slot; wkT dead -> expS_A; (wqT -> wo? no:
    # wqT slot hosts wo NEXT rep; this rep wo goes into the m slot)
    expS_A = big.tile([P, SKT // 2, SQ], BF, tag="wkT")
    expS_B = big.tile([P, SKT // 2, SQ], BF, tag="wqT")
    wo_s = big.tile([P, DT, D], FP16, tag="m")
    nc.sync.dma_start(wo_s[:], wo_d.rearrange("(t p) f -> p t f", p=P))

    def _expS(skt):
        return expS_A[:, skt] if skt < SKT // 2 else expS_B[:, skt - SKT // 2]

    # ---- 3. S^T = X Q'^T : out [sk-tile, sq-chunk], accumulate over d'
    for skt in range(SKT):
        pss = [psmm.tile([P, NCH], F32, tag="mm", name=f"ps{i}") for i in range(QCH)]
        for dt, ch in [(d, c) for c in range(QCH) for d in range(DT)]:
            nc.tensor.matmul(
                pss[ch][:],
                xt_s[:, dt, skt * P : (skt + 1) * P],
                qt2_s[:, dt, ch * NCH : (ch + 1) * NCH],
                start=(dt == 0),
                stop=(dt == DT - 1),
            )
        for ch in range(QCH):
            nc.scalar.activation(
                _expS(skt)[:, ch * NCH : (ch + 1) * NCH],
                pss[ch][:],
                mybir.ActivationFunctionType.Exp,
                scale=SCALE,
            )

    # ---- 4. row sums + reciprocal broadcast
    ones = small.tile([P, 1], BF)
    nc.any.memset(ones[:], 1.0)
    row = small.tile([1, SQ], F32)
    for ch in range(QCH):
        psr = psrow.tile([1, NCH], F32, tag="row")
        for skt in range(SKT):
            nc.tensor.matmul(
                psr[:],
                ones[:],
                _expS(skt)[:, ch * NCH : (ch + 1) * NCH],
                start=(skt == 0),
                stop=(skt == SKT - 1),
            )
        nc.any.tensor_copy(out=row[:, ch * NCH : (ch + 1) * NCH], in_=psr[:])
    rrow = small.tile([1, SQ], F32)
    ones1 = small.tile([1, P], F32)
    nc.any.memset(ones1[:], 1.0)
    recip_rep = small.tile([P, SQ], F32)

    def _recip_block():
        nc.vector.reciprocal(rrow[:], row[:])
        for ch in range(QCH):
            psr2 = psmm.tile([P, NCH], F32, tag="mm", name="psrep")
            nc.tensor.matmul(
                psr2[:],
                ones1[:],
                rrow[:, ch * NCH : (ch + 1) * NCH],
                start=True,
                stop=True,
            )
            nc.any.tensor_copy(
                out=recip_rep[:, ch * NCH : (ch + 1) * NCH], in_=psr2[:]
            )

    # ---- 5. U = X^T expS : out [d-tile, sq-chunk], accumulate over keys
    u_s = big.tile([P, DT, SQ], BF, tag="qt2")
    for dtl in range(DT):
        pss = [psmm.tile([P, NCH], F32, tag="mm", name=f"ps{i}") for i in range(QCH)]
        for skt, ch in [(s_, c) for c in range(QCH) for s_ in range(SKT)]:
            nc.tensor.matmul(
                pss[ch][:],
                xn_s[:, skt, dtl * P : (dtl + 1) * P],
                _expS(skt)[:, ch * NCH : (ch + 1) * NCH],
                start=(skt == 0),
                stop=(skt == SKT - 1),
            )
        if dtl == 0:
            _recip_block()
        for ch in range(QCH):
            nc.any.tensor_copy(
                out=u_s[:, dtl, ch * NCH : (ch + 1) * NCH], in_=pss[ch][:]
            )

    # ---- 6. O^T = Wv^T U : out [e'-tile, sq-chunk], accumulate over d;
    # 1/den fused at the PSUM->SBUF multiply
    ot_s = big.tile([P, DT, SQ], FP16, tag="ot")
    for et in range(DT):
        pss = [psmm.tile([P, NCH], F32, tag="mm", name=f"ps{i}") for i in range(QCH)]
        for dt, ch in [(d, c) for c in range(QCH) for d in range(DT)]:
            nc.tensor.matmul(
                pss[ch][:],
                wv_s[:, dt, et * P : (et + 1) * P],
                u_s[:, dt, ch * NCH : (ch + 1) * NCH],
                start=(dt == 0),
                stop=(dt == DT - 1),
            )
        for ch in range(QCH):
            nc.vector.tensor_mul(
                out=ot_s[:, et, ch * NCH : (ch + 1) * NCH],
                in0=pss[ch][:],
                in1=recip_rep[:, ch * NCH : (ch + 1) * NCH],
            )

    # ---- 7. Y = O^T^T Wo
    SQT = SQ // P
    FCH = D // NCH
    for sqt in range(SQT):
        pss = [psmm.tile([P, NCH], F32, tag="mm", name=f"ps{i}") for i in range(FCH)]
        for et, ch in [(e, c) for c in range(FCH) for e in range(DT)]:
            nc.tensor.matmul(
                pss[ch][:],
                ot_s[:, et, sqt * P : (sqt + 1) * P],
                wo_s[:, et, ch * NCH : (ch + 1) * NCH],
                start=(et == 0),
                stop=(et == DT - 1),
            )
        y_stage = yst.tile([P, D], F32, tag="y")
        for ch in range(FCH):
            nc.any.tensor_copy(out=y_stage[:, ch * NCH : (ch + 1) * NCH], in_=pss[ch][:])
        nc.sync.dma_start(y_d[sqt * P : (sqt + 1) * P, :], y_stage[:])


# ---------------------------------------------------------------------------
# PJRT runner (axon): jit once per process, chain `reps` executions.
# ---------------------------------------------------------------------------

def _make_runner(nc, n_cores, reps=1):
    from concourse.bass2jax import (
        _bass_exec_p,
        install_neuronx_cc_hook,
        partition_id_tensor,
    )
    from jax.sharding import Mesh, PartitionSpec
    from jax.experimental.shard_map import shard_map

    install_neuronx_cc_hook()
    partition_name = nc.partition_id_tensor.name if nc.partition_id_tensor else None

    in_names, out_names, out_avals, zero_outs = [], [], [], []
    for alloc in nc.m.functions[0].allocations:
        if not isinstance(alloc, mybir.MemoryLocationSet):
            continue
        name = alloc.memorylocations[0].name
        if alloc.kind == "ExternalInput":
            if name != partition_name:
                in_names.append(name)
        elif alloc.kind == "ExternalOutput":
            shape = tuple(alloc.tensor_shape)
            dtype = mybir.dt.np(alloc.dtype)
            out_names.append(name)
            out_avals.append(jax.core.ShapedArray(shape, dtype))
            zero_outs.append(np.zeros(shape, dtype))
    n_params = len(in_names)
    n_outs = len(out_avals)
    all_in_names = list(in_names) + list(out_names)
    if partition_name is not None:
        all_in_names.append(partition_name)

    def _body(*args):
        operands = list(args)
        pid = [partition_id_tensor()] if partition_name is not None else []
        outs = None
        for _ in range(reps):
            outs = _bass_exec_p.bind(
                *operands,
                *pid,
                out_avals=tuple(out_avals),
                in_names=tuple(all_in_names),
                out_names=tuple(out_names),
                lowering_input_output_aliases=(),
                sim_require_finite=True,
                sim_require_nnan=True,
                nc=nc,
            )
        return tuple(outs)

    devices = jax.devices()[:n_cores]
    mesh = Mesh(np.asarray(devices), ("core",))
    in_specs = (PartitionSpec("core"),) * (n_params + n_outs)
    out_specs = (PartitionSpec("core"),) * n_outs
    fn = jax.jit(
        shard_map(
            _body, mesh=mesh, in_specs=in_specs, out_specs=out_specs, check_rep=False
        )
    )

    def run(in_maps):
        per_core = [[np.asarray(m[name]) for name in in_names] for m in in_maps]
        concat_in = [
            np.ascontiguousarray(
                np.concatenate([per_core[c][i] for c in range(n_cores)], axis=0)
            )
            for i in range(n_params)
        ]
        concat_zeros = [
            np.zeros((n_cores * z.shape[0], *z.shape[1:]), z.dtype) for z in zero_outs
        ]
        out_arrs = fn(*concat_in, *concat_zeros)
        jax.block_until_ready(out_arrs)
        return [
            {
                name: np.asarray(out_arrs[i]).reshape(n_cores, *out_avals[i].shape)[c]
                for i, name in enumerate(out_names)
            }
            for c in range(n_cores)
        ]

    run.fn = fn
    run.in_names = in_names
    run.out_names = out_names
    run.zero_outs = zero_outs
    run.n_cores = n_cores
    return run


_CACHE = {}


def _get_runner(reps=1):
    key = ("runner", reps)
    if key not in _CACHE:
        _CACHE[key] = _make_runner(_build(reps=reps), NCORES)
    return _CACHE[key]


def _prep_in_maps(inputs, Wq, Wk, Wv, Wo):
    f16 = np.float16
    w = {
        "wqt": np.ascontiguousarray(np.asarray(Wq).T).astype(f16),
        "wkt": np.ascontiguousarray(np.asarray(Wk).T).astype(f16),
        "wv": np.ascontiguousarray(Wv).astype(f16),
        "wo": np.ascontiguousarray(Wo).astype(f16),
    }
    in_maps = []
    for c in range(NCORES):
        b, h = divmod(c, 2)
        xb = np.asarray(inputs[b])
        if h == 1:
            # own queries first; keys rotated identically in both layouts
            xb = np.concatenate([xb[SQ:], xb[:SQ]], axis=0)
        in_maps.append(
            {
                "xt": np.ascontiguousarray(xb.T).astype(f16),
                "xn": np.ascontiguousarray(xb).astype(f16),
                **w,
            }
        )
    return in_maps


def kernel(inputs, Wq, Wk, Wv, Wo):
    inputs = np.asarray(inputs, dtype=np.float32)
    run = _get_runner()
    in_maps = _prep_in_maps(inputs, Wq, Wk, Wv, Wo)
    res = run(in_maps)
    out = np.empty((B, S, D), dtype=np.float32)
    for c in range(NCORES):
        b, h = divmod(c, 2)
        out[b, h * SQ : (h + 1) * SQ] = res[c]["y"]
    return out


# revision 16
# speedup vs baseline: 1.0185x; 1.0001x over previous
"""MHA v5: algebraic K/V-projection elimination. No collectives.

Identities (per batch):
  S = (X Wq)(X Wk)^T = X M X^T,  M = Wq Wk^T   (M is seq-independent!)
  O = P (X Wv) = (P X) Wv = (U^T Wv),  U = X^T P^T  (contract keys first)

Per-core phases (own 1024 queries; keys rotated so own queries come first):
  1. M     [d,d']  = Wq Wk^T        : lhsT=wqT slices, rhs=wkT     128 mm
  2. Q'^T  [d',sq] = M^T X^T        : lhsT=m slices,   rhs=xt      128 mm
  3. S^T   [sk,sq] = X Q'^T         : lhsT=xt slices,  rhs=qt2     256 mm
     + exp (ACT, scale 1/8 fused) -> expS bf16
  4. rowsum (ones-matmul) + reciprocal broadcast
  5. U     [d,sq]  = X^T expS       : lhsT=xnat slices, rhs=expS   256 mm
  6. O^T   [e',sq] = Wv^T U         : lhsT=wv slices,  rhs=u       128 mm
     normalized by 1/den at the PSUM->SBUF multiply
  7. Y     [sq,f]  = O^T^T Wo       : lhsT=ot slices,  rhs=wo      128 mm

1024 big matmuls/core vs 1312 for the direct form (-22% PE work).
Host sends xt=X^T, xnat=X (both key-rotated for h=1), wqT=Wq^T, wkT=Wk^T,
wv, wo. All DMA lines >= 1KB.
"""

import numpy as np

import jax

import concourse.mybir as mybir
import concourse.tile as tile
from concourse import bacc


P = 128
D = 1024
S = 2048
SQ = 1024
B = 4
NCORES = 8
DT = D // P  # 8
SKT = S // P  # 16
NCH = 512
QCH = SQ // NCH  # 2
BF = mybir.dt.bfloat16
FP16 = mybir.dt.float16
F32 = mybir.dt.float32
SCALE = 0.125


def _build(reps=1, loop_reps=None):
    nc = bacc.Bacc("TRN2", debug=False, enable_asserts=False, num_devices=NCORES)

    xt_d = nc.dram_tensor("xt", [D, S], FP16, kind="ExternalInput").ap()
    xn_d = nc.dram_tensor("xn", [S, D], FP16, kind="ExternalInput").ap()
    wqt_d = nc.dram_tensor("wqt", [D, D], FP16, kind="ExternalInput").ap()
    wkt_d = nc.dram_tensor("wkt", [D, D], FP16, kind="ExternalInput").ap()
    wv_d = nc.dram_tensor("wv", [D, D], FP16, kind="ExternalInput").ap()
    wo_d = nc.dram_tensor("wo", [D, D], FP16, kind="ExternalInput").ap()
    y_d = nc.dram_tensor("y", [SQ, D], F32, kind="ExternalOutput").ap()

    with tile.TileContext(nc) as tc:
        with (
            tc.tile_pool(name="big", bufs=1) as big,
            tc.tile_pool(name="yst", bufs=2) as yst,
            tc.tile_pool(name="small", bufs=1) as small,
            tc.tile_pool(name="psmm", bufs=6, space="PSUM") as psmm,
            tc.tile_pool(name="psrow", bufs=2, space="PSUM") as psrow,
        ):
            import contextlib

            # timing loops run 2 bodies per HW-loop iteration: halves the
            # loop-edge cost and lets consecutive executions overlap as
            # straight-line dataflow
            if loop_reps:
                assert loop_reps % 2 == 0
                loop_ctx = tc.For_i(0, loop_reps // 2, 1)
                body_n = 2
            else:
                loop_ctx = contextlib.nullcontext()
                body_n = reps
            with loop_ctx:
                for _rep in range(body_n):
                    _body(nc, tc, big, yst, small, psmm, psrow,
                          xt_d, xn_d, wqt_d, wkt_d, wv_d, wo_d, y_d)

    nc.compile()
    return nc


def _body(nc, tc, big, yst, small, psmm, psrow,
          xt_d, xn_d, wqt_d, wkt_d, wv_d, wo_d, y_d):
    # ---- SBUF slots (per-partition KB): xt 32, xnat 32, wqT 16 (->wo),
    # wkT 16 (->expS_A), m 16 (->expS_B), qt2 16 (->u), wv 16, ot 16 = 160
    xt_s = big.tile([P, DT, S], FP16, tag="xt")
    xn_s = big.tile([P, SKT, D], FP16, tag="xn")
    wqt_s = big.tile([P, DT, D], FP16, tag="wqT")
    wkt_s = big.tile([P, DT, D], FP16, tag="wkT")
    m_s = big.tile([P, DT, D], FP16, tag="m")
    qt2_s = big.tile([P, DT, SQ], FP16, tag="qt2")
    wv_s = big.tile([P, DT, D], FP16, tag="wv")

    xt_r = xt_d.rearrange("(t p) s -> p t s", p=P)
    xn_r = xn_d.rearrange("(t p) d -> p t d", p=P)

    # weights first (M is the first phase), split in M-consumption order so
    # the first M group gates on ~2MB not 4MB; then xt (2KB lines), xnat
    wqt_r = wqt_d.rearrange("(t p) d -> p t d", p=P)
    wkt_r = wkt_d.rearrange("(t p) d -> p t d", p=P)
    HD = D // 2
    nc.sync.dma_start(wqt_s[:, :, 0:HD], wqt_r[:, :, 0:HD])
    nc.sync.dma_start(wkt_s[:, :, 0:HD], wkt_r[:, :, 0:HD])
    nc.sync.dma_start(wkt_s[:, :, HD:D], wkt_r[:, :, HD:D])
    nc.sync.dma_start(wqt_s[:, :, HD:D], wqt_r[:, :, HD:D])
    for cc in range(2):
        for dt in range(DT):
            nc.sync.dma_start(
                xt_s[:, dt, cc * SQ : (cc + 1) * SQ],
                xt_r[:, dt, cc * SQ : (cc + 1) * SQ],
            )
    nc.sync.dma_start(wv_s[:], wv_d.rearrange("(t p) e -> p t e", p=P))
    nc.sync.dma_start(xn_s[:], xn_r)

    # ---- 1. M = Wq Wk^T : out [d-tile, d'-chunk], accumulate over e
    for dtl in range(DT):
        pss = [psmm.tile([P, NCH], F32, tag="mm", name=f"ps{i}") for i in range(QCH)]
        for et, ch in [(e, c) for c in range(QCH) for e in range(DT)]:
            nc.tensor.matmul(
                pss[ch][:],
                wqt_s[:, et, dtl * P : (dtl + 1) * P],
                wkt_s[:, et, ch * NCH : (ch + 1) * NCH],
                start=(et == 0),
                stop=(et == DT - 1),
            )
        for ch in range(QCH):
            nc.any.tensor_copy(
                out=m_s[:, dtl, ch * NCH : (ch + 1) * NCH], in_=pss[ch][:]
            )

    # ---- 2. Q'^T = M^T X^T : out [d'-tile, sq-chunk], accumulate over d
    for dtl in range(DT):
        pss = [psmm.tile([P, NCH], F32, tag="mm", name=f"ps{i}") for i in range(QCH)]
        for dt, ch in [(d, c) for c in range(QCH) for d in range(DT)]:
            nc.tensor.matmul(
                pss[ch][:],
                m_s[:, dt, dtl * P : (dtl + 1) * P],
                xt_s[:, dt, ch * NCH : (ch + 1) * NCH],
                start=(dt == 0),
                stop=(dt == DT - 1),
            )
        for ch in range(QCH):
            nc.any.tensor_copy(
                out=qt2_s[:, dtl, ch * NCH : (ch + 1) * NCH], in_=pss[ch][:]
            )

    # m dead -> wo loads into its slot; wkT dead -> expS_A; (wqT -> wo? no:
    # wqT slot hosts wo NEXT rep; this rep wo goes into the m slot)
    expS_A = big.tile([P, SKT // 2, SQ], BF, tag="wkT")
    expS_B = big.tile([P, SKT // 2, SQ], BF, tag="wqT")
    wo_s = big.tile([P, DT, D], FP16, tag="m")
    nc.sync.dma_start(wo_s[:], wo_d.rearrange("(t p) f -> p t f", p=P))

    def _expS(skt):
        return expS_A[:, skt] if skt < SKT // 2 else expS_B[:, skt - SKT // 2]

    esum = small.tile([P, SQ], F32, name="esum")

    # ---- 3. S^T = X Q'^T : out [sk-tile, sq-chunk], accumulate over d'
    for skt in range(SKT):
        pss = [psmm.tile([P, NCH], F32, tag="mm", name=f"ps{i}") for i in range(QCH)]
        for dt, ch in [(d, c) for c in range(QCH) for d in range(DT)]:
            nc.tensor.matmul(
                pss[ch][:],
                xt_s[:, dt, skt * P : (skt + 1) * P],
                qt2_s[:, dt, ch * NCH : (ch + 1) * NCH],
                start=(dt == 0),
                stop=(dt == DT - 1),
            )
        for ch in range(QCH):
            nc.scalar.activation(
                _expS(skt)[:, ch * NCH : (ch + 1) * NCH],
                pss[ch][:],
                mybir.ActivationFunctionType.Exp,
                scale=SCALE,
            )
        # denominator partials on DVE (hidden under the S phase) instead of
        # 32 ones-matmuls on the PE critical path
        if skt == 0:
            nc.vector.tensor_copy(out=esum[:], in_=_expS(0)[:])
        else:
            nc.vector.tensor_add(out=esum[:], in0=esum[:], in1=_expS(skt)[:])

    # ---- 4. partition-reduce the DVE partial sums + reciprocal broadcast
    ones = small.tile([P, 1], BF)
    nc.any.memset(ones[:], 1.0)
    esum_bf = small.tile([P, SQ], BF, name="esum_bf")
    nc.any.tensor_copy(out=esum_bf[:], in_=esum[:])
    row = small.tile([1, SQ], F32)
    for ch in range(QCH):
        psr = psrow.tile([1, NCH], F32, tag="row")
        nc.tensor.matmul(
            psr[:],
            ones[:],
            esum_bf[:, ch * NCH : (ch + 1) * NCH],
            start=True,
            stop=True,
        )
        nc.any.tensor_copy(out=row[:, ch * NCH : (ch + 1) * NCH], in_=psr[:])
    rrow = small.tile([1, SQ], F32)
    ones1 = small.tile([1, P], F32)
    nc.any.memset(ones1[:], 1.0)
    recip_rep = small.tile([P, SQ], F32)

    def _recip_block():
        nc.vector.reciprocal(rrow[:], row[:])
        for ch in range(QCH):
            psr2 = psmm.tile([P, NCH], F32, tag="mm", name="psrep")
            nc.tensor.matmul(
                psr2[:],
                ones1[:],
                rrow[:, ch * NCH : (ch + 1) * NCH],
                start=True,
                stop=True,
            )
            nc.any.tensor_copy(
                out=recip_rep[:, ch * NCH : (ch + 1) * NCH], in_=psr2[:]
            )

    # ---- 5. U = X^T expS : out [d-tile, sq-chunk], accumulate over keys
    u_s = big.tile([P, DT, SQ], BF, tag="qt2")
    for dtl in range(DT):
        pss = [psmm.tile([P, NCH], F32, tag="mm", name=f"ps{i}") for i in range(QCH)]
        for skt, ch in [(s_, c) for c in range(QCH) for s_ in range(SKT)]:
            nc.tensor.matmul(
                pss[ch][:],
                xn_s[:, skt, dtl * P : (dtl + 1) * P],
                _expS(skt)[:, ch * NCH : (ch + 1) * NCH],
                start=(skt == 0),
                stop=(skt == SKT - 1),
            )
        if dtl == 0:
            _recip_block()
        for ch in range(QCH):
            nc.any.tensor_copy(
                out=u_s[:, dtl, ch * NCH : (ch + 1) * NCH], in_=pss[ch][:]
            )

    # ---- 6. O^T = Wv^T U : out [e'-tile, sq-chunk], accumulate over d;
    # 1/den fused at the PSUM->SBUF multiply
    ot_s = big.tile([P, DT, SQ], FP16, tag="ot")
    for et in range(DT):
        pss = [psmm.tile([P, NCH], F32, tag="mm", name=f"ps{i}") for i in range(QCH)]
        for dt, ch in [(d, c) for c in range(QCH) for d in range(DT)]:
            nc.tensor.matmul(
                pss[ch][:],
                wv_s[:, dt, et * P : (et + 1) * P],
                u_s[:, dt, ch * NCH : (ch + 1) * NCH],
                start=(dt == 0),
                stop=(dt == DT - 1),
            )
        for ch in range(QCH):
            nc.vector.tensor_mul(
                out=ot_s[:, et, ch * NCH : (ch + 1) * NCH],
                in0=pss[ch][:],
                in1=recip_rep[:, ch * NCH : (ch + 1) * NCH],
            )

    # ---- 7. Y = O^T^T Wo
    SQT = SQ // P
    FCH = D // NCH
    for sqt in range(SQT):
        pss = [psmm.tile([P, NCH], F32, tag="mm", name=f"ps{i}") for i in range(FCH)]
        for et, ch in [(e, c) for c in range(FCH) for e in range(DT)]:
            nc.tensor.matmul(
                pss[ch][:],
                ot_s[:, et, sqt * P : (sqt + 1) * P],
                wo_s[:, et, ch * NCH : (ch + 1) * NCH],
                start=(et == 0),
                stop=(et == DT - 1),
            )
        y_stage = yst.tile([P, D], F32, tag="y")
        for ch in range(FCH):
            nc.any.tensor_copy(out=y_stage[:, ch * NCH : (ch + 1) * NCH], in_=pss[ch][:])
        nc.sync.dma_start(y_d[sqt * P : (sqt + 1) * P, :], y_stage[:])


# ---------------------------------------------------------------------------
# PJRT runner (axon): jit once per process, chain `reps` executions.
# ---------------------------------------------------------------------------

def _make_runner(nc, n_cores, reps=1):
    from concourse.bass2jax import (
        _bass_exec_p,
        install_neuronx_cc_hook,
        partition_id_tensor,
    )
    from jax.sharding import Mesh, PartitionSpec
    from jax.experimental.shard_map import shard_map

    install_neuronx_cc_hook()
    partition_name = nc.partition_id_tensor.name if nc.partition_id_tensor else None

    in_names, out_names, out_avals, zero_outs = [], [], [], []
    for alloc in nc.m.functions[0].allocations:
        if not isinstance(alloc, mybir.MemoryLocationSet):
            continue
        name = alloc.memorylocations[0].name
        if alloc.kind == "ExternalInput":
            if name != partition_name:
                in_names.append(name)
        elif alloc.kind == "ExternalOutput":
            shape = tuple(alloc.tensor_shape)
            dtype = mybir.dt.np(alloc.dtype)
            out_names.append(name)
            out_avals.append(jax.core.ShapedArray(shape, dtype))
            zero_outs.append(np.zeros(shape, dtype))
    n_params = len(in_names)
    n_outs = len(out_avals)
    all_in_names = list(in_names) + list(out_names)
    if partition_name is not None:
        all_in_names.append(partition_name)

    def _body(*args):
        operands = list(args)
        pid = [partition_id_tensor()] if partition_name is not None else []
        outs = None
        for _ in range(reps):
            outs = _bass_exec_p.bind(
                *operands,
                *pid,
                out_avals=tuple(out_avals),
                in_names=tuple(all_in_names),
                out_names=tuple(out_names),
                lowering_input_output_aliases=(),
                sim_require_finite=True,
                sim_require_nnan=True,
                nc=nc,
            )
        return tuple(outs)

    devices = jax.devices()[:n_cores]
    mesh = Mesh(np.asarray(devices), ("core",))
    in_specs = (PartitionSpec("core"),) * (n_params + n_outs)
    out_specs = (PartitionSpec("core"),) * n_outs
    fn = jax.jit(
        shard_map(
            _body, mesh=mesh, in_specs=in_specs, out_specs=out_specs, check_rep=False
        )
    )

    def run(in_maps):
        per_core = [[np.asarray(m[name]) for name in in_names] for m in in_maps]
        concat_in = [
            np.ascontiguousarray(
                np.concatenate([per_core[c][i] for c in range(n_cores)], axis=0)
            )
            for i in range(n_params)
        ]
        concat_zeros = [
            np.zeros((n_cores * z.shape[0], *z.shape[1:]), z.dtype) for z in zero_outs
        ]
        out_arrs = fn(*concat_in, *concat_zeros)
        jax.block_until_ready(out_arrs)
        return [
            {
                name: np.asarray(out_arrs[i]).reshape(n_cores, *out_avals[i].shape)[c]
                for i, name in enumerate(out_names)
            }
            for c in range(n_cores)
        ]

    run.fn = fn
    run.in_names = in_names
    run.out_names = out_names
    run.zero_outs = zero_outs
    run.n_cores = n_cores
    return run


_CACHE = {}


def _get_runner(reps=1):
    key = ("runner", reps)
    if key not in _CACHE:
        _CACHE[key] = _make_runner(_build(reps=reps), NCORES)
    return _CACHE[key]


def _prep_in_maps(inputs, Wq, Wk, Wv, Wo):
    f16 = np.float16
    w = {
        "wqt": np.ascontiguousarray(np.asarray(Wq).T).astype(f16),
        "wkt": np.ascontiguousarray(np.asarray(Wk).T).astype(f16),
        "wv": np.ascontiguousarray(Wv).astype(f16),
        "wo": np.ascontiguousarray(Wo).astype(f16),
    }
    in_maps = []
    for c in range(NCORES):
        b, h = divmod(c, 2)
        xb = np.asarray(inputs[b])
        if h == 1:
            # own queries first; keys rotated identically in both layouts
            xb = np.concatenate([xb[SQ:], xb[:SQ]], axis=0)
        in_maps.append(
            {
                "xt": np.ascontiguousarray(xb.T).astype(f16),
                "xn": np.ascontiguousarray(xb).astype(f16),
                **w,
            }
        )
    return in_maps


def kernel(inputs, Wq, Wk, Wv, Wo):
    inputs = np.asarray(inputs, dtype=np.float32)
    run = _get_runner()
    in_maps = _prep_in_maps(inputs, Wq, Wk, Wv, Wo)
    res = run(in_maps)
    out = np.empty((B, S, D), dtype=np.float32)
    for c in range(NCORES):
        b, h = divmod(c, 2)
        out[b, h * SQ : (h + 1) * SQ] = res[c]["y"]
    return out
